# revision 6
# baseline (speedup 1.0000x reference)
"""Distributed Trainium2 (Bass/Tile) kernel for a batched quantized matmul.

Reference computation (all shapes hardcoded):
    out[s,b,m,n] = sum_k (x[s,b,m,k] + 66)*0.03 * (y[b,k,n] - 160)*0.025
    x: [7, 8, 1024, 1024] f32 holding ints in [-128, 127]
    y: [8, 1024, 1024]    f32 holding ints in [0, 255]
    out: [7, 8, 1024, 1024] f32

Sharding: data-parallel over B=8 -> one batch element b per NeuronCore.
Core b gets x[:, b] and y[b]; no collectives needed.

Device kernel (per core), fp8 DoubleRow variant:
  - The rel-err gate is 2e-2; quantizing the zero-point-shifted operands
    (x+66 in [-62,193], y-160 in [-160,95]) to TRN fp8e4 (e4m3, max 240)
    costs 4.8e-3 rel err (validated in numpy AND on hw) -- well inside
    the gate. Host pre-applies the zero points during the fp8 cast, so
    the device does no dequant arithmetic at all; the combined scale
    0.03*0.025 = 7.5e-4 is fused into the PSUM->SBUF eviction.
  - fp8e4 matmuls in DoubleRow mode contract 256 k-elements per
    instruction (2 multiplies/cell/cycle): half the bf16 instruction
    count for the same work. 448 MMs x 213ns = 95.5us PE floor; the
    kernel streams them back-to-back at that rate (measured).
  - Plain DoubleRow ran MMs at 259ns: the 256-column non-contiguous
    LDWEIGHTS stole the rhs stream's SBUF/XBUS bandwidth. With
    DoubleRowSwInterleave the host pre-interleaves each weight tile
    into one contiguous 256B/partition block; LDWEIGHTS (130ns) then
    overlaps 100% and MMs hit the 213ns roofline.
  - Startup (trace-measured): the runtime preamble gates the first DMA
    issue to ~7.3us; the PE HAM clock gate holds 1.2GHz until the
    trailing activity window fills (~17us in the baseline). The first
    2MB of operands are bandwidth-bound: each DMA ring sustains only
    ~170GB/s while two rings contend. So: y rides the sync HWDGE ring
    as 8x128KB per-(ki,nj) tiles, s=0 x rides the scalar HWDGE ring
    (idle until its first eviction ~15us; first chunk split
    64KB/192KB), and the s>=1 x prefetch on the gpsimd SWDGE ring is
    held back by a data dependency on the last s=0 chunk so it cannot
    steal startup bandwidth (s=1 x isn't consumed until ~14us after
    the first MM, so the delay is free). First real MM ~9.8us vs
    11.7us with everything contending.
  - Warm-up dummy matmuls bridge PE from its preamble (~7.6us) to the
    first operand arrival so the HAM activity window keeps filling and
    there is no PE idle gap that would push the 2.4GHz ramp later.
  - Eviction alternates ScalarE/DVE per stripe and store issues ride
    the sync queue: one queue cannot hold 57 x 1.26us evictions plus
    57 x 0.7us dma_start issue slots inside the PE span.
  - Output is stored bf16 (halves out-DMA; +2e-4 rel err) and upcast
    to f32 on the host.
  - Tail: the final stripe's two half-evictions drain on both evictor
    engines and their two store issues ride different queues
    (scalar + sync) so the issues don't serialize.
"""

import numpy as np
import ml_dtypes

import concourse.bass as bass
import concourse.mybir as mybir
from concourse import bacc
from concourse.tile import TileContext
from concourse.bass_utils import run_bass_kernel_spmd

S, B, M, K, N = 7, 8, 1024, 1024, 1024
P = 128          # SBUF partitions / PE array dim
NB = 512         # one PSUM bank of fp32
KP = 2 * P       # k-elements contracted per DoubleRow matmul
KTT, MTT = K // KP, M // P  # 4, 8 (host-side tiling of the x layout)
X_ZP = -66.0
Y_ZP = 160.0
OUT_SCALE = 0.03 * 0.025
BF16 = mybir.dt.bfloat16
FP8 = mybir.dt.float8e4
F32 = mybir.dt.float32
ACT_COPY = mybir.ActivationFunctionType.Copy
DR_SW = mybir.MatmulPerfMode.DoubleRowSwInterleave

_CACHED_NC = None


def build():
    # Bacc (not plain Bass): its finalize() runs generate_event_semaphores,
    # which splits multi-wait sync_info to the <=1-wait-per-instruction HW
    # limit (walrus rejects the unsplit form with "Too many sync waits").
    nc = bacc.Bacc("TRN2", target_bir_lowering=False)
    KT, MT, NT = K // KP, M // P, N // NB  # 4, 8, 2
    # x is provided per (s, ki2) in DoubleRowSwInterleave weight layout:
    # x_d[s, ki2, p, mj*256 + 2*j + i] = xq[m = mj*128 + 127 - j,
    #                                       k = ki2*256 + i*128 + p]
    # so each weight tile is one contiguous 256B/partition LDWEIGHTS read.
    x_d = nc.declare_dram_parameter("x", [S, KT, P, MT * 2 * P], FP8,
                                    isOutput=False)
    # y is provided pre-tiled per (ki2, nj): y_d[ki2, nj, p, i, n'] =
    # yq[ki2*256+i*128+p, nj*512+n'] -- 8 contiguous [128, 1024B] 2D DMAs.
    # Finer tiles than per-ki2 so the first matmul's rhs (128KB) lands as
    # early as possible on the startup-bandwidth-bound sync ring.
    y_d = nc.declare_dram_parameter("y", [KT, NT, P, 2, NB], FP8,
                                    isOutput=False)
    o_d = nc.declare_dram_parameter("out", [S, M, N], BF16, isOutput=True)
    # 4-byte scratch sink for the prefetch-gate DMA (see below); never
    # read by the host.
    g_d = nc.declare_dram_parameter("scratch", [1, 4], FP8, isOutput=True)

    with TileContext(nc) as tc:
        with tc.tile_pool(name="ypool", bufs=1) as ypool, \
             tc.tile_pool(name="xpool", bufs=2 * KT) as xpool, \
             tc.tile_pool(name="pspool", bufs=4, space="PSUM") as pspool, \
             tc.tile_pool(name="opool", bufs=6) as opool:
            # Warm-up: the PE HAM clock gate holds the array at 1.2 GHz
            # until its trailing activity window fills. Burn the DMA wait
            # on dummy matmuls so PE activity is continuous from the
            # preamble to the first real MM. Only one column is memset
            # (tile allocation needs a producer); the rest is read as
            # garbage, which is fine: the PE has no traps, the warm PSUM
            # bank is never read, and the first real matmul's start=True
            # resets it. 4 x 512-col + 1 x 256-col at the cold 1.2GHz
            # clock spans ~7.6..9.9us, bridging to operand arrival ~9.8us.
            warm_src = ypool.tile([P, NB], BF16, tag="warmsrc")
            nc.vector.memset(warm_src[:, 0:1], 1.0)
            warm_ps = pspool.tile([P, N], F32, tag="ps", name="warm")
            for _ in range(4):
                nc.tensor.matmul(warm_ps[:, 0:NB], warm_src[:, 0:P],
                                 warm_src[:], start=True, stop=True)
            nc.tensor.matmul(warm_ps[:, 0:NB // 2], warm_src[:, 0:P],
                             warm_src[:, 0:NB // 2], start=True, stop=True)

            # Startup loads. Two HWDGE rings in parallel (each ~170GB/s
            # while both pull): y as 8 per-(ki,nj) 128KB tiles on the sync
            # ring; s=0 x as 5 chunks on the scalar ring. Scalar's 5
            # dma_start issue slots precede its auto-inserted
            # ACT_TABLE_LOAD and first eviction (~15us) in program order,
            # so they fire at ~7.3us sharp. The first x chunk carries only
            # mj0-1 (64KB) so the first LDWEIGHTS fires ~9.5us.
            yq = [[None] * NT for _ in range(KT)]
            for ki in range(KT):
                for nj in range(NT):
                    yt = ypool.tile([P, 2, NB], FP8, tag=f"y{ki}{nj}")
                    nc.sync.dma_start(out=yt[:], in_=y_d[ki, nj])
                    yq[ki][nj] = yt
            xT0 = [None] * KT
            for ki in range(KT):
                xt = xpool.tile([P, MT, 2 * P], FP8, tag="xT", name="xt0")
                if ki == 0:
                    nc.scalar.dma_start(out=xt[:, 0:2, :],
                                        in_=x_d[0, 0][:, 0:2 * 2 * P])
                    nc.scalar.dma_start(out=xt[:, 2:MT, :],
                                        in_=x_d[0, 0][:, 2 * 2 * P:])
                else:
                    nc.scalar.dma_start(out=xt[:], in_=x_d[0, ki])
                xT0[ki] = xt

            # Hold the s>=1 x prefetch (gpsimd SWDGE ring) until the last
            # s=0 chunk has landed: a 4-byte gpsimd store reading xT0[3]
            # makes every later gpsimd dma_start queue behind that DMA's
            # completion. s=1 x isn't consumed until ~14us after the
            # first MM, so this only removes startup ring contention.
            nc.gpsimd.dma_start(out=g_d[:], in_=xT0[KT - 1][0:1, 0, 0:4])

            def evict(ot_sl, ps_sl, odd):
                # PSUM -> SBUF bf16 with fused scale, alternating between
                # the Scalar and Vector engines so neither eviction queue
                # accumulates backlog against the PE stream (a single queue
                # carrying all 57 x ~1.26us evictions plus issue overhead
                # runs within ~5% of the whole kernel span).
                if odd:
                    nc.vector.tensor_scalar_mul(ot_sl, ps_sl, OUT_SCALE)
                else:
                    nc.scalar.activation(ot_sl, ps_sl, ACT_COPY,
                                         scale=OUT_SCALE)

            def store(dram_sl, ot_sl, odd, queue=None):
                # store issues ride the near-idle sync queue: the ~0.7us
                # dma_start sequencer cost plus the ~0.75us cross-queue
                # wait fit easily there, and the store is off the
                # PSUM-recycle critical path (it only reads the SBUF copy)
                (queue or nc.sync).dma_start(out=dram_sl, in_=ot_sl)

            def mj_group(s, mj, xT, odd, split_evict=False):
                """One output stripe [128, 1024]: ki-inner accumulation into
                a 2-bank PSUM tile, then a single eviction + store. For the
                very last group, evict/store per nj half instead so the nj=0
                half drains while nj=1's final matmuls still stream, and the
                two store issues ride different queues (scalar is idle by
                then) so they don't serialize on sync."""
                pst = pspool.tile([P, N], F32, tag="ps", name="ps")
                for ki in range(KT):
                    lhsT = xT[ki][:, mj, :]
                    for nj in range(NT):
                        nc.tensor.matmul(
                            pst[:, nj * NB:(nj + 1) * NB], lhsT,
                            yq[ki][nj][:], start=(ki == 0), stop=(ki == KT - 1),
                            perf_mode=DR_SW)
                ot = opool.tile([P, N], BF16, tag="o", name="ot")
                if split_evict:
                    # last stripe: drain the two nj halves on the two
                    # evictor queues in parallel
                    for nj in range(NT):
                        sl = slice(nj * NB, (nj + 1) * NB)
                        evict(ot[:, sl], pst[:, sl], nj % 2)
                        store(o_d[s, mj * P:(mj + 1) * P, sl], ot[:, sl],
                              nj % 2,
                              queue=(nc.scalar if nj == 0 else nc.sync))
                else:
                    evict(ot[:], pst[:], odd)
                    store(o_d[s, mj * P:(mj + 1) * P, :], ot[:], odd)

            for s in range(S):
                if s == 0:
                    xT = xT0
                    # Startup: operands arrive at DMA rate; consume each ki
                    # chunk for two mj stripes as it lands (ki-outer, 2 open
                    # groups -- same interleaving degree as the plain loop).
                    # For ki=0 the nj loop is outermost so the first two MMs
                    # need only y(0,0) and the 64KB mj0-1 x chunk.
                    MJ_HEAD = 2
                    head = [pspool.tile([P, N], F32, tag="ps", name=f"ph{mj}")
                            for mj in range(MJ_HEAD)]
                    for nj in range(NT):
                        for mj in range(MJ_HEAD):
                            nc.tensor.matmul(
                                head[mj][:, nj * NB:(nj + 1) * NB],
                                xT0[0][:, mj, :], yq[0][nj][:],
                                start=True, stop=False, perf_mode=DR_SW)
                    for ki in range(1, KT):
                        for mj in range(MJ_HEAD):
                            lhsT = xT[ki][:, mj, :]
                            for nj in range(NT):
                                nc.tensor.matmul(
                                    head[mj][:, nj * NB:(nj + 1) * NB], lhsT,
                                    yq[ki][nj][:],
                                    start=False, stop=(ki == KT - 1),
                                    perf_mode=DR_SW)
                    for mj in range(MJ_HEAD):
                        ot = opool.tile([P, N], BF16, tag="o", name="oth")
                        evict(ot[:], head[mj][:], mj % 2)
                        store(o_d[0, mj * P:(mj + 1) * P, :], ot[:], mj % 2)
                    for mj in range(MJ_HEAD, MT):
                        mj_group(s, mj, xT, mj % 2)
                    continue
                else:
                    xT = []
                    for ki in range(KT):
                        xt = xpool.tile([P, MT, 2 * P], FP8, tag="xT")
                        nc.gpsimd.dma_start(out=xt[:], in_=x_d[s, ki])
                        xT.append(xt)
                for mj in range(MT):
                    mj_group(s, mj, xT, mj % 2,
                             split_evict=(s == S - 1 and mj == MT - 1))
    nc.finalize()
    return nc


def _shard_inputs(x, y):
    f8 = ml_dtypes.float8_e4m3
    in_maps = []
    for b in range(B):
        # zero points pre-applied; |values| <= 193 fit e4m3 (max 240)
        # with <= 6.25% per-element rounding error -> ~4.6e-3 rel err.
        # x shard: k-major transpose, then the DoubleRowSwInterleave weight
        # layout (see build()): per (s, ki2, mj) block of 256, position
        # 2*j + i holds column (127 - j) of k-subtile i.
        xq = (np.ascontiguousarray(x[:, b].transpose(0, 2, 1))
              - np.float32(X_ZP)).astype(f8)          # [S, K, M]
        a = xq.reshape(S, KTT, 2, P, MTT, P)          # [s, ki2, i, p, mj, j]
        a = a.transpose(0, 1, 3, 4, 5, 2)[:, :, :, :, ::-1, :]
        # y: per-(ki2, nj) DoubleRow tile layout [ki2, nj, p, i, n'] (one
        # contiguous 1KB/partition 2D DMA per tile)
        yq = (y[b] - np.float32(Y_ZP)).astype(f8)    # [K, N]
        yq = yq.reshape(KTT, 2, P, 2, NB).transpose(0, 3, 2, 1, 4)
        in_maps.append({
            "x": np.ascontiguousarray(a).reshape(S, KTT, P, MTT * 2 * P),
            "y": np.ascontiguousarray(yq),
        })
    return in_maps


def run(x, y, trace=False):
    global _CACHED_NC
    if _CACHED_NC is None:
        _CACHED_NC = build()
    nc = _CACHED_NC
    in_maps = _shard_inputs(x, y)
    res = run_bass_kernel_spmd(nc, in_maps, core_ids=list(range(B)), trace=trace)
    out = np.stack([np.asarray(res.results[b]["out"]) for b in range(B)], axis=1)
    return out.astype(np.float32), res


def kernel(x, y):
    out, _ = run(x, y, trace=False)
    return out


# revision 13
# speedup vs baseline: 1.1548x; 1.1548x over previous
"""Distributed Trainium2 (Bass/Tile) kernel for a batched quantized matmul.

Reference computation (all shapes hardcoded):
    out[s,b,m,n] = sum_k (x[s,b,m,k] + 66)*0.03 * (y[b,k,n] - 160)*0.025
    x: [7, 8, 1024, 1024] f32 holding ints in [-128, 127]
    y: [8, 1024, 1024]    f32 holding ints in [0, 255]
    out: [7, 8, 1024, 1024] f32

Sharding: data-parallel over B=8 -> one batch element b per NeuronCore.
Core b gets x[:, b] and y[b]; no collectives needed.

Device kernel (per core), fp8 DoubleRow variant:
  - The rel-err gate is 2e-2; quantizing the zero-point-shifted operands
    (x+66 in [-62,193], y-160 in [-160,95]) to TRN fp8e4 (e4m3, max 240)
    costs 4.8e-3 rel err (validated in numpy AND on hw) -- well inside
    the gate. Host pre-applies the zero points during the fp8 cast, so
    the device does no dequant arithmetic at all; the combined scale
    0.03*0.025 = 7.5e-4 is fused into the PSUM->SBUF eviction.
  - fp8e4 matmuls in DoubleRow mode contract 256 k-elements per
    instruction (2 multiplies/cell/cycle): half the bf16 instruction
    count for the same work. 448 MMs x 213ns = 95.5us PE floor; the
    kernel streams them back-to-back at that rate (measured).
  - Plain DoubleRow ran MMs at 259ns: the 256-column non-contiguous
    LDWEIGHTS stole the rhs stream's SBUF/XBUS bandwidth. With
    DoubleRowSwInterleave the host pre-interleaves each weight tile
    into one contiguous 256B/partition block; LDWEIGHTS (130ns) then
    overlaps 100% and MMs hit the 213ns roofline.
  - Startup (trace-measured): the runtime preamble gates the first DMA
    issue to ~7.3us; the PE HAM clock gate holds 1.2GHz until the
    trailing activity window fills (~17us in the baseline). The first
    2MB of operands are bandwidth-bound: each DMA ring sustains only
    ~170GB/s while two rings contend. So: y rides the sync HWDGE ring
    as 4x256KB per-ki tiles, s=0 and s=1 x ride the scalar HWDGE ring
    (idle until its first eviction ~15us; s=0's first chunk split
    64KB/192KB so the first LDWEIGHTS fires early; s=1's descriptors
    queue behind s=0's in ring order, off the startup window), and the
    s>=2 x prefetch on the gpsimd SWDGE ring is naturally held back by
    xpool recycling (waits on s=0's last LDWEIGHTS ~23us). First real
    MM ~9.8us vs 11.7us with everything contending.
  - Warm-up dummy matmuls bridge PE from its preamble (~7.6us) to the
    first operand arrival so the HAM activity window keeps filling and
    there is no PE idle gap that would push the 2.4GHz ramp later.
  - Eviction alternates ScalarE/DVE per stripe and store issues ride
    the sync queue: one queue cannot hold 57 x 1.26us evictions plus
    57 x 0.7us dma_start issue slots inside the PE span.
  - Output is stored bf16 (halves out-DMA; +2e-4 rel err) and upcast
    to f32 on the host.
  - Tail: the final stripe's two half-evictions drain on both evictor
    engines and their two store issues ride different queues
    (scalar + sync) so the issues don't serialize.
"""

import numpy as np
import ml_dtypes

import concourse.bass as bass
import concourse.mybir as mybir
from concourse import bacc
from concourse.tile import TileContext
from concourse.bass_utils import run_bass_kernel_spmd

S, B, M, K, N = 7, 8, 1024, 1024, 1024
P = 128          # SBUF partitions / PE array dim
NB = 512         # one PSUM bank of fp32
KP = 2 * P       # k-elements contracted per DoubleRow matmul
KTT, MTT = K // KP, M // P  # 4, 8 (host-side tiling of the x layout)
X_ZP = -66.0
Y_ZP = 160.0
OUT_SCALE = 0.03 * 0.025
BF16 = mybir.dt.bfloat16
FP8 = mybir.dt.float8e4
F32 = mybir.dt.float32
ACT_COPY = mybir.ActivationFunctionType.Copy
DR_SW = mybir.MatmulPerfMode.DoubleRowSwInterleave

_CACHED_NC = None


def build():
    # Bacc (not plain Bass): its finalize() runs generate_event_semaphores,
    # which splits multi-wait sync_info to the <=1-wait-per-instruction HW
    # limit (walrus rejects the unsplit form with "Too many sync waits").
    nc = bacc.Bacc("TRN2", target_bir_lowering=False)
    KT, MT, NT = K // KP, M // P, N // NB  # 4, 8, 2
    # x is provided per (s, ki2) in DoubleRowSwInterleave weight layout:
    # x_d[s, ki2, p, mj*256 + 2*j + i] = xq[m = mj*128 + 127 - j,
    #                                       k = ki2*256 + i*128 + p]
    # so each weight tile is one contiguous 256B/partition LDWEIGHTS read.
    x_d = nc.declare_dram_parameter("x", [S, KT, P, MT * 2 * P], FP8,
                                    isOutput=False)
    # y is provided pre-tiled per ki2: y_d[ki2, p, i, n] = yq[ki2*256+i*128+p, n]
    # so each y tile is a single contiguous [128, 2048B] DMA. NOTE: tiling
    # y finer (per-(ki,nj) [128,2,512] tiles) was measured 20us SLOWER:
    # with the i-row stride at 512B instead of 1024B every matmul's rhs
    # stream throttles (259ns/MM, the plain-DoubleRow penalty rate).
    y_d = nc.declare_dram_parameter("y", [KT, P, 2, N], FP8, isOutput=False)
    o_d = nc.declare_dram_parameter("out", [S, M, N], BF16, isOutput=True)

    with TileContext(nc) as tc:
        with tc.tile_pool(name="ypool", bufs=1) as ypool, \
             tc.tile_pool(name="xpool", bufs=2 * KT) as xpool, \
             tc.tile_pool(name="pspool", bufs=4, space="PSUM") as pspool, \
             tc.tile_pool(name="opool", bufs=6) as opool:
            # Warm-up: the PE HAM clock gate holds the array at 1.2 GHz
            # until its trailing activity window fills. Burn the DMA wait
            # on dummy matmuls so PE activity is continuous from the
            # preamble to the first real MM. Only one column is memset
            # (tile allocation needs a producer); the rest is read as
            # garbage, which is fine: the PE has no traps, the warm PSUM
            # bank is never read, and the first real matmul's start=True
            # resets it. 4 x 512-col + 1 x 256-col at the cold 1.2GHz
            # clock spans ~7.6..9.9us, bridging to operand arrival ~9.8us.
            warm_src = ypool.tile([P, NB], BF16, tag="warmsrc")
            nc.vector.memset(warm_src[:, 0:1], 1.0)
            warm_ps = pspool.tile([P, N], F32, tag="ps", name="warm")
            for _ in range(4):
                nc.tensor.matmul(warm_ps[:, 0:NB], warm_src[:, 0:P],
                                 warm_src[:], start=True, stop=True)
            nc.tensor.matmul(warm_ps[:, 0:NB // 2], warm_src[:, 0:P],
                             warm_src[:, 0:NB // 2], start=True, stop=True)

            # Startup loads. Two HWDGE rings in parallel (each ~170GB/s
            # while both pull): y as 4 per-ki 256KB tiles on the sync
            # ring; s=0 AND s=1 x as chunks on the scalar ring. Scalar's
            # dma_start issue slots precede its auto-inserted
            # ACT_TABLE_LOAD and first eviction (~15us) in program order,
            # so they fire at ~7.4us sharp; s=1's descriptors queue
            # behind s=0's on the same ring, so they cannot steal startup
            # bandwidth and still land by ~20us (s=1 is consumed from
            # ~26us). The first x chunk carries only mj0-1 (64KB) so the
            # first LDWEIGHTS fires ~9.5us. s>=2 x rides the gpsimd
            # SWDGE ring, naturally held back by xpool recycling (its
            # dma_start waits for the s=0 tiles' last LDWEIGHTS ~23us).
            yq = [None] * KT
            for ki in range(KT):
                yt = ypool.tile([P, 2, N], FP8, tag=f"y{ki}")
                nc.sync.dma_start(out=yt[:], in_=y_d[ki])
                yq[ki] = yt
            xT0 = [None] * KT
            for ki in range(KT):
                xt = xpool.tile([P, MT, 2 * P], FP8, tag="xT", name="xt0")
                if ki == 0:
                    nc.scalar.dma_start(out=xt[:, 0:2, :],
                                        in_=x_d[0, 0][:, 0:2 * 2 * P])
                    nc.scalar.dma_start(out=xt[:, 2:MT, :],
                                        in_=x_d[0, 0][:, 2 * 2 * P:])
                else:
                    nc.scalar.dma_start(out=xt[:], in_=x_d[0, ki])
                xT0[ki] = xt

            def evict(ot_sl, ps_sl, odd):
                # PSUM -> SBUF bf16 with fused scale, alternating between
                # the Scalar and Vector engines so neither eviction queue
                # accumulates backlog against the PE stream (a single queue
                # carrying all 57 x ~1.26us evictions plus issue overhead
                # runs within ~5% of the whole kernel span).
                if odd:
                    nc.vector.tensor_scalar_mul(ot_sl, ps_sl, OUT_SCALE)
                else:
                    nc.scalar.activation(ot_sl, ps_sl, ACT_COPY,
                                         scale=OUT_SCALE)

            def store(dram_sl, ot_sl, odd, queue=None):
                # store issues ride the near-idle sync queue: the ~0.7us
                # dma_start sequencer cost plus the ~0.75us cross-queue
                # wait fit easily there, and the store is off the
                # PSUM-recycle critical path (it only reads the SBUF copy)
                (queue or nc.sync).dma_start(out=dram_sl, in_=ot_sl)

            def mj_group(s, mj, xT, odd, split_evict=False):
                """One output stripe [128, 1024]: ki-inner accumulation into
                a 2-bank PSUM tile, then a single eviction + store. For the
                very last group, evict/store per nj half instead so the nj=0
                half drains while nj=1's final matmuls still stream, and the
                two store issues ride different queues (scalar is idle by
                then) so they don't serialize on sync."""
                pst = pspool.tile([P, N], F32, tag="ps", name="ps")
                for ki in range(KT):
                    lhsT = xT[ki][:, mj, :]
                    for nj in range(NT):
                        nc.tensor.matmul(
                            pst[:, nj * NB:(nj + 1) * NB], lhsT,
                            yq[ki][:, :, nj * NB:(nj + 1) * NB],
                            start=(ki == 0), stop=(ki == KT - 1),
                            perf_mode=DR_SW)
                ot = opool.tile([P, N], BF16, tag="o", name="ot")
                if split_evict:
                    # last stripe: drain the two nj halves on the two
                    # evictor queues in parallel
                    for nj in range(NT):
                        sl = slice(nj * NB, (nj + 1) * NB)
                        evict(ot[:, sl], pst[:, sl], nj % 2)
                        store(o_d[s, mj * P:(mj + 1) * P, sl], ot[:, sl],
                              nj % 2,
                              queue=(nc.scalar if nj == 0 else nc.sync))
                else:
                    evict(ot[:], pst[:], odd)
                    store(o_d[s, mj * P:(mj + 1) * P, :], ot[:], odd)

            for s in range(S):
                if s == 0:
                    xT = xT0
                    # Startup: operands arrive at DMA rate; consume each ki
                    # chunk for two mj stripes as it lands (ki-outer, 2 open
                    # groups -- same interleaving degree as the plain loop).
                    # For ki=0 the nj loop is outermost so the first two MMs
                    # need only y(0,0) and the 64KB mj0-1 x chunk.
                    MJ_HEAD = 2
                    head = [pspool.tile([P, N], F32, tag="ps", name=f"ph{mj}")
                            for mj in range(MJ_HEAD)]
                    for ki in range(KT):
                        for mj in range(MJ_HEAD):
                            lhsT = xT[ki][:, mj, :]
                            for nj in range(NT):
                                nc.tensor.matmul(
                                    head[mj][:, nj * NB:(nj + 1) * NB], lhsT,
                                    yq[ki][:, :, nj * NB:(nj + 1) * NB],
                                    start=(ki == 0), stop=(ki == KT - 1),
                                    perf_mode=DR_SW)
                    for mj in range(MJ_HEAD):
                        ot = opool.tile([P, N], BF16, tag="o", name="oth")
                        evict(ot[:], head[mj][:], mj % 2)
                        store(o_d[0, mj * P:(mj + 1) * P, :], ot[:], mj % 2)
                    for mj in range(MJ_HEAD, MT):
                        mj_group(s, mj, xT, mj % 2)
                    continue
                else:
                    # s=1 rides the scalar HWDGE ring behind s=0's chunks
                    # (in-ring descriptor order keeps it off the startup
                    # window); s>=2 rides gpsimd, gated by xpool recycling.
                    q = nc.scalar if s == 1 else nc.gpsimd
                    xT = []
                    for ki in range(KT):
                        xt = xpool.tile([P, MT, 2 * P], FP8, tag="xT")
                        q.dma_start(out=xt[:], in_=x_d[s, ki])
                        xT.append(xt)
                for mj in range(MT):
                    mj_group(s, mj, xT, mj % 2,
                             split_evict=(s == S - 1 and mj == MT - 1))
    nc.finalize()
    return nc


def _shard_inputs(x, y):
    f8 = ml_dtypes.float8_e4m3
    in_maps = []
    for b in range(B):
        # zero points pre-applied; |values| <= 193 fit e4m3 (max 240)
        # with <= 6.25% per-element rounding error -> ~4.6e-3 rel err.
        # x shard: k-major transpose, then the DoubleRowSwInterleave weight
        # layout (see build()): per (s, ki2, mj) block of 256, position
        # 2*j + i holds column (127 - j) of k-subtile i.
        xq = (np.ascontiguousarray(x[:, b].transpose(0, 2, 1))
              - np.float32(X_ZP)).astype(f8)          # [S, K, M]
        a = xq.reshape(S, KTT, 2, P, MTT, P)          # [s, ki2, i, p, mj, j]
        a = a.transpose(0, 1, 3, 4, 5, 2)[:, :, :, :, ::-1, :]
        # y: per-ki2 DoubleRow tile layout [ki2, p, i, n] (one DMA per tile)
        yq = (y[b] - np.float32(Y_ZP)).astype(f8)    # [K, N]
        yq = yq.reshape(KTT, 2, P, N).transpose(0, 2, 1, 3)
        in_maps.append({
            "x": np.ascontiguousarray(a).reshape(S, KTT, P, MTT * 2 * P),
            "y": np.ascontiguousarray(yq),
        })
    return in_maps


def run(x, y, trace=False):
    global _CACHED_NC
    if _CACHED_NC is None:
        _CACHED_NC = build()
    nc = _CACHED_NC
    in_maps = _shard_inputs(x, y)
    res = run_bass_kernel_spmd(nc, in_maps, core_ids=list(range(B)), trace=trace)
    out = np.stack([np.asarray(res.results[b]["out"]) for b in range(B)], axis=1)
    return out.astype(np.float32), res


def kernel(x, y):
    out, _ = run(x, y, trace=False)
    return out


# revision 18
# speedup vs baseline: 1.2043x; 1.0429x over previous
"""Distributed Trainium2 (Bass/Tile) kernel for a batched quantized matmul.

Reference computation (all shapes hardcoded):
    out[s,b,m,n] = sum_k (x[s,b,m,k] + 66)*0.03 * (y[b,k,n] - 160)*0.025
    x: [7, 8, 1024, 1024] f32 holding ints in [-128, 127]
    y: [8, 1024, 1024]    f32 holding ints in [0, 255]
    out: [7, 8, 1024, 1024] f32

Sharding: data-parallel over B=8 -> one batch element b per NeuronCore.
Core b gets x[:, b] and y[b]; no collectives needed.

Device kernel (per core), fp8 DoubleRow variant:
  - The rel-err gate is 2e-2; quantizing the zero-point-shifted operands
    (x+66 in [-62,193], y-160 in [-160,95]) to TRN fp8e4 (e4m3, max 240)
    costs 4.8e-3 rel err (validated in numpy AND on hw) -- well inside
    the gate. Host pre-applies the zero points during the fp8 cast, so
    the device does no dequant arithmetic at all; the combined scale
    0.03*0.025 = 7.5e-4 is fused into the PSUM->SBUF eviction.
  - fp8e4 matmuls in DoubleRow mode contract 256 k-elements per
    instruction (2 multiplies/cell/cycle): half the bf16 instruction
    count for the same work. 448 MMs x 213ns = 95.5us PE floor; the
    kernel streams them back-to-back at that rate (measured).
  - Plain DoubleRow ran MMs at 259ns: the 256-column non-contiguous
    LDWEIGHTS stole the rhs stream's SBUF/XBUS bandwidth. With
    DoubleRowSwInterleave the host pre-interleaves each weight tile
    into one contiguous 256B/partition block; LDWEIGHTS (130ns) then
    overlaps 100% and MMs hit the 213ns roofline.
  - Startup (trace-measured): the runtime preamble gates the first DMA
    issue to ~7.3us; the PE HAM clock gate holds 1.2GHz until its
    trailing activity window fills, and every PE idle gap pushes the
    2.4GHz ramp later. The first 2MB of operands are bandwidth-bound:
    y rides the sync HWDGE ring (4x256KB), s=0 x rides the gpsimd
    SWDGE ring (the two ring types pull in parallel; two HWDGE rings
    instead SHARE one ~180GB/s budget -- measured 4us slower), with
    the x chunks reordered so the 3 head stripes' weights for all ki
    land first. Real MMs start ~10.2us and never stall, dummy warm-up
    matmuls keep PE busy from 7.9us, and the clock ramp lands ~4us
    earlier than with a gappy start.
  - Warm-up dummy matmuls bridge PE from its preamble (~7.6us) to the
    first operand arrival so the HAM activity window keeps filling and
    there is no PE idle gap that would push the 2.4GHz ramp later.
  - Eviction alternates ScalarE/DVE per stripe and store issues ride
    the sync queue: one queue cannot hold 57 x 1.26us evictions plus
    57 x 0.7us dma_start issue slots inside the PE span.
  - Output is stored bf16 (halves out-DMA; +2e-4 rel err) and upcast
    to f32 on the host.
  - Tail: the final stripe's two half-evictions drain on both evictor
    engines and their two store issues ride different queues
    (scalar + sync) so the issues don't serialize.
"""

import numpy as np
import ml_dtypes

import concourse.bass as bass
import concourse.mybir as mybir
from concourse import bacc
from concourse.tile import TileContext
from concourse.bass_utils import run_bass_kernel_spmd

S, B, M, K, N = 7, 8, 1024, 1024, 1024
P = 128          # SBUF partitions / PE array dim
NB = 512         # one PSUM bank of fp32
KP = 2 * P       # k-elements contracted per DoubleRow matmul
KTT, MTT = K // KP, M // P  # 4, 8 (host-side tiling of the x layout)
X_ZP = -66.0
Y_ZP = 160.0
OUT_SCALE = 0.03 * 0.025
BF16 = mybir.dt.bfloat16
FP8 = mybir.dt.float8e4
F32 = mybir.dt.float32
ACT_COPY = mybir.ActivationFunctionType.Copy
DR_SW = mybir.MatmulPerfMode.DoubleRowSwInterleave

_CACHED_NC = None


def build():
    # Bacc (not plain Bass): its finalize() runs generate_event_semaphores,
    # which splits multi-wait sync_info to the <=1-wait-per-instruction HW
    # limit (walrus rejects the unsplit form with "Too many sync waits").
    nc = bacc.Bacc("TRN2", target_bir_lowering=False)
    KT, MT, NT = K // KP, M // P, N // NB  # 4, 8, 2
    # x is provided per (s, ki2) in DoubleRowSwInterleave weight layout:
    # x_d[s, ki2, p, mj*256 + 2*j + i] = xq[m = mj*128 + 127 - j,
    #                                       k = ki2*256 + i*128 + p]
    # so each weight tile is one contiguous 256B/partition LDWEIGHTS read.
    x_d = nc.declare_dram_parameter("x", [S, KT, P, MT * 2 * P], FP8,
                                    isOutput=False)
    # y is provided pre-tiled per ki2: y_d[ki2, p, i, n] = yq[ki2*256+i*128+p, n]
    # so each y tile is a single contiguous [128, 2048B] DMA. NOTE: tiling
    # y finer (per-(ki,nj) [128,2,512] tiles) was measured 20us SLOWER:
    # with the i-row stride at 512B instead of 1024B every matmul's rhs
    # stream throttles (259ns/MM, the plain-DoubleRow penalty rate).
    y_d = nc.declare_dram_parameter("y", [KT, P, 2, N], FP8, isOutput=False)
    o_d = nc.declare_dram_parameter("out", [S, M, N], BF16, isOutput=True)

    with TileContext(nc) as tc:
        with tc.tile_pool(name="ypool", bufs=1) as ypool, \
             tc.tile_pool(name="xpool", bufs=2 * KT) as xpool, \
             tc.tile_pool(name="pspool", bufs=4, space="PSUM") as pspool, \
             tc.tile_pool(name="opool", bufs=6) as opool:
            # Warm-up: the PE HAM clock gate holds the array at 1.2 GHz
            # until its trailing activity window fills (~5.3us busy in a
            # ~7.6us window); any PE idle gap pushes the 2.4GHz ramp
            # later, at half-rate matmuls all the while. Burn the DMA
            # wait on dummy matmuls so PE activity is continuous from
            # the preamble (~7.9us) to the first operand arrival
            # (~10.2us). Only one column is memset (tile allocation
            # needs a producer); the rest is read as garbage, which is
            # fine: the PE has no traps, the warm PSUM bank is never
            # read, and the first real matmul's start=True resets it.
            warm_src = ypool.tile([P, NB], BF16, tag="warmsrc")
            nc.vector.memset(warm_src[:, 0:1], 1.0)
            warm_ps = pspool.tile([P, N], F32, tag="ps", name="warm")
            for _ in range(5):
                nc.tensor.matmul(warm_ps[:, 0:NB], warm_src[:, 0:P],
                                 warm_src[:], start=True, stop=True)

            # Startup loads. The sync HWDGE ring and the gpsimd SWDGE
            # ring pull in parallel (~180 + ~160 GB/s; two HWDGE rings
            # instead SHARE ~180GB/s -- measured 4us slower): y as 4
            # per-ki 256KB tiles on sync; s=0 x on gpsimd, reordered so
            # the 3 head stripes' weights for ALL ki (4 x 96KB) land
            # first (~10.2, 10.9, 11.5, 12.2us), then the mj3-7
            # remainders (4 x 160KB). The 3-stripe head consumes a ki
            # chunk per 2.55us at the cold clock, so the stream never
            # stalls while the remainders arrive. s>=1 x also rides
            # gpsimd: its descriptors queue behind s=0's 8 (in-ring
            # order + ~6us of issue slots), and s>=2 is additionally
            # held back by xpool recycling (waits on s=0's last
            # LDWEIGHTS ~24us), so the startup window stays clean.
            MJ_HEAD = 3
            yq = [None] * KT
            for ki in range(KT):
                yt = ypool.tile([P, 2, N], FP8, tag=f"y{ki}")
                nc.sync.dma_start(out=yt[:], in_=y_d[ki])
                yq[ki] = yt
            xT0 = [None] * KT
            for ki in range(KT):
                xt = xpool.tile([P, MT, 2 * P], FP8, tag="xT", name="xt0")
                nc.gpsimd.dma_start(out=xt[:, 0:MJ_HEAD, :],
                                    in_=x_d[0, ki][:, 0:MJ_HEAD * 2 * P])
                xT0[ki] = xt
            for ki in range(KT):
                nc.gpsimd.dma_start(out=xT0[ki][:, MJ_HEAD:MT, :],
                                    in_=x_d[0, ki][:, MJ_HEAD * 2 * P:])

            def evict(ot_sl, ps_sl, odd):
                # PSUM -> SBUF bf16 with fused scale, alternating between
                # the Scalar and Vector engines so neither eviction queue
                # accumulates backlog against the PE stream (a single queue
                # carrying all 57 x ~1.26us evictions plus issue overhead
                # runs within ~5% of the whole kernel span).
                if odd:
                    nc.vector.tensor_scalar_mul(ot_sl, ps_sl, OUT_SCALE)
                else:
                    nc.scalar.activation(ot_sl, ps_sl, ACT_COPY,
                                         scale=OUT_SCALE)

            def store(dram_sl, ot_sl, odd, queue=None):
                # store issues ride the near-idle sync queue: the ~0.7us
                # dma_start sequencer cost plus the ~0.75us cross-queue
                # wait fit easily there, and the store is off the
                # PSUM-recycle critical path (it only reads the SBUF copy)
                (queue or nc.sync).dma_start(out=dram_sl, in_=ot_sl)

            def mj_group(s, mj, xT, odd, split_evict=False):
                """One output stripe [128, 1024]: ki-inner accumulation into
                a 2-bank PSUM tile, then a single eviction + store. For the
                very last group, evict/store per nj half instead so the nj=0
                half drains while nj=1's final matmuls still stream, and the
                two store issues ride different queues (scalar is idle by
                then) so they don't serialize on sync."""
                pst = pspool.tile([P, N], F32, tag="ps", name="ps")
                for ki in range(KT):
                    lhsT = xT[ki][:, mj, :]
                    for nj in range(NT):
                        nc.tensor.matmul(
                            pst[:, nj * NB:(nj + 1) * NB], lhsT,
                            yq[ki][:, :, nj * NB:(nj + 1) * NB],
                            start=(ki == 0), stop=(ki == KT - 1),
                            perf_mode=DR_SW)
                ot = opool.tile([P, N], BF16, tag="o", name="ot")
                if split_evict:
                    # last stripe: drain the two nj halves on the two
                    # evictor queues in parallel
                    for nj in range(NT):
                        sl = slice(nj * NB, (nj + 1) * NB)
                        evict(ot[:, sl], pst[:, sl], nj % 2)
                        store(o_d[s, mj * P:(mj + 1) * P, sl], ot[:, sl],
                              nj % 2,
                              queue=(nc.scalar if nj == 0 else nc.sync))
                else:
                    evict(ot[:], pst[:], odd)
                    store(o_d[s, mj * P:(mj + 1) * P, :], ot[:], odd)

            for s in range(S):
                if s == 0:
                    xT = xT0
                    # Startup: operands arrive at DMA rate; consume each ki
                    # chunk for the three head stripes as it lands
                    # (ki-outer, 3 open groups matching the 96KB head
                    # chunks, fed at the arrival cadence).
                    head = [pspool.tile([P, N], F32, tag="ps", name=f"ph{mj}")
                            for mj in range(MJ_HEAD)]
                    for ki in range(KT):
                        for mj in range(MJ_HEAD):
                            lhsT = xT[ki][:, mj, :]
                            for nj in range(NT):
                                nc.tensor.matmul(
                                    head[mj][:, nj * NB:(nj + 1) * NB], lhsT,
                                    yq[ki][:, :, nj * NB:(nj + 1) * NB],
                                    start=(ki == 0), stop=(ki == KT - 1),
                                    perf_mode=DR_SW)
                    for mj in range(MJ_HEAD):
                        ot = opool.tile([P, N], BF16, tag="o", name="oth")
                        evict(ot[:], head[mj][:], mj % 2)
                        store(o_d[0, mj * P:(mj + 1) * P, :], ot[:], mj % 2)
                    for mj in range(MJ_HEAD, MT):
                        mj_group(s, mj, xT, mj % 2)
                    continue
                else:
                    xT = []
                    for ki in range(KT):
                        xt = xpool.tile([P, MT, 2 * P], FP8, tag="xT")
                        nc.gpsimd.dma_start(out=xt[:], in_=x_d[s, ki])
                        xT.append(xt)
                for mj in range(MT):
                    mj_group(s, mj, xT, mj % 2,
                             split_evict=(s == S - 1 and mj == MT - 1))
    nc.finalize()
    return nc


def _shard_inputs(x, y):
    f8 = ml_dtypes.float8_e4m3
    in_maps = []
    for b in range(B):
        # zero points pre-applied; |values| <= 193 fit e4m3 (max 240)
        # with <= 6.25% per-element rounding error -> ~4.6e-3 rel err.
        # x shard: k-major transpose, then the DoubleRowSwInterleave weight
        # layout (see build()): per (s, ki2, mj) block of 256, position
        # 2*j + i holds column (127 - j) of k-subtile i.
        xq = (np.ascontiguousarray(x[:, b].transpose(0, 2, 1))
              - np.float32(X_ZP)).astype(f8)          # [S, K, M]
        a = xq.reshape(S, KTT, 2, P, MTT, P)          # [s, ki2, i, p, mj, j]
        a = a.transpose(0, 1, 3, 4, 5, 2)[:, :, :, :, ::-1, :]
        # y: per-ki2 DoubleRow tile layout [ki2, p, i, n] (one DMA per tile)
        yq = (y[b] - np.float32(Y_ZP)).astype(f8)    # [K, N]
        yq = yq.reshape(KTT, 2, P, N).transpose(0, 2, 1, 3)
        in_maps.append({
            "x": np.ascontiguousarray(a).reshape(S, KTT, P, MTT * 2 * P),
            "y": np.ascontiguousarray(yq),
        })
    return in_maps


def run(x, y, trace=False):
    global _CACHED_NC
    if _CACHED_NC is None:
        _CACHED_NC = build()
    nc = _CACHED_NC
    in_maps = _shard_inputs(x, y)
    res = run_bass_kernel_spmd(nc, in_maps, core_ids=list(range(B)), trace=trace)
    out = np.stack([np.asarray(res.results[b]["out"]) for b in range(B)], axis=1)
    return out.astype(np.float32), res


def kernel(x, y):
    out, _ = run(x, y, trace=False)
    return out
